# revision 10
# baseline (speedup 1.0000x reference)
"""Trainium2 Bass kernel for nn_FEPModel (byte-LM with sliding-window attention
blocks around a delta-rule chunk-memory module).

Sharding: 8 cores = 4 batches x 2 sequence halves. Each core holds 1152 tokens
(128-token left halo + 1024 resident), runs the full model on its shard.
Cross-core traffic: 3 pairwise halo AllGathers (0.5 MB) between SWA blocks and
one 8-way AllGather of the 4 chunk means (16 KB) for the memory stage.

Layouts: residual stream token-major [128, 9, 1024] f32 in SBUF. LayerNorm via
bn_stats (free-dim reduce). LN gain/bias folded into the following linear's
weights on host; v-projection bias folded into out-proj bias (softmax rows sum
to 1). Matmuls in bf16 with f32 PSUM accumulation. Feature-major linear biases
are fused into PSUM-eviction activations as per-partition [128,1] operands;
token-major biases are added from DMA-broadcast rows.
"""

import numpy as np

try:
    import concourse.bass as bass
except ImportError:
    import sys
    sys.path.insert(0, "/opt/trn_rl_repo")
    import concourse.bass as bass

import ml_dtypes
import concourse.bacc as bacc
import concourse.mybir as mybir
import concourse.tile as tile
import concourse.bass_utils as bass_utils
from concourse.bass import ts
from concourse.masks import make_identity

F32 = mybir.dt.float32
BF16 = mybir.dt.bfloat16
NPBF16 = ml_dtypes.bfloat16
AF = mybir.ActivationFunctionType
ALU = mybir.AluOpType
AX = mybir.AxisListType

B, S, D = 4, 2048, 1024
NH, DH = 4, D // 4
CS, DK, MH = 256, 64, 3
EPS = 1e-5
NCORES = 8
TRES, TALL, NT = 1024, 1152, 9
NBLK = 4

_CACHE = {}


def _bcast_row(ap, parts):
    """DRAM [1, N] row -> partition-broadcast AP [parts, N]."""
    return bass.AP(tensor=ap.tensor, offset=ap.offset,
                   ap=[[0, parts]] + list(ap.ap[1:]))


# ---------------------------------------------------------------- device build
def _declare_inputs(nc):
    d = {}

    def di(name, shape, dt):
        d[name] = nc.dram_tensor(name, list(shape), dt, kind="ExternalInput").ap()

    di("emb_hi", (256, D), BF16)
    di("emb_lo", (256, D), BF16)
    di("oh", (256, TALL), BF16)
    di("pos", (TALL, D), F32)
    di("masks", (2, 128, 256), F32)
    di("sel", (NT, 32, 128), F32)
    di("cones", (8, 128, 4), F32)
    for i in range(NBLK):
        di(f"w_qkv{i}", (D, 3 * D), BF16)
        di(f"b_qkv{i}", (16, 128), F32)     # q,k bias, partition-major
        di(f"w_out{i}", (D, D), BF16)
        di(f"b_out{i}", (1, D), F32)        # includes folded v bias
        di(f"w_ff1{i}", (D, 3 * D), BF16)
        di(f"b_ff1{i}", (24, 128), F32)     # partition-major
        di(f"w_ff2{i}", (3 * D, D), BF16)
        di(f"b_ff2{i}", (1, D), F32)
    di("mln_g", (1, D), F32)
    di("mln_b", (1, D), F32)
    di("mq_w", (MH, D, DK), BF16)
    di("mq_b", (1, MH * DK), F32)
    di("mprec_w", (D, MH * D), BF16)
    di("mprec_b", (1, MH * D), F32)
    di("mS", (DK, MH * D), BF16)
    di("m_gw_ow", (3 * D, D), BF16)         # rows 0:2048 gate_w, 2048:3072 outp_w
    di("mgate_b", (1, D), F32)
    di("moutp_b", (1, D), F32)
    di("head_w", (D, 256), BF16)
    di("head_b", (1, 256), F32)
    return d


def _build():
    nc = bacc.Bacc("TRN2", target_bir_lowering=False, debug=False,
                   enable_asserts=False, num_devices=NCORES)
    I = _declare_inputs(nc)
    logits_ap = nc.dram_tensor("logits", [TRES, 256], F32,
                               kind="ExternalOutput").ap()
    fe_ap = nc.dram_tensor("fe", [1, 1], F32, kind="ExternalOutput").ap()

    with tile.TileContext(nc) as tc:
        _program(nc, tc, I, logits_ap, fe_ap)
    nc.compile()
    return nc


def _program(nc, tc, I, logits_ap, fe_ap):
    import contextlib
    ctx = contextlib.ExitStack()
    with ctx:
        pers = ctx.enter_context(tc.tile_pool(name="pers", bufs=1))
        consts = ctx.enter_context(tc.tile_pool(name="consts", bufs=1))
        wbig = ctx.enter_context(tc.tile_pool(name="wbig", bufs=1))
        wsmall = ctx.enter_context(tc.tile_pool(name="wsmall", bufs=1))
        work = ctx.enter_context(tc.tile_pool(name="work", bufs=2))
        stat = ctx.enter_context(tc.tile_pool(name="stat", bufs=4))
        bbp = ctx.enter_context(tc.tile_pool(name="bbp", bufs=1))
        psA = ctx.enter_context(tc.tile_pool(name="psA", bufs=2, space="PSUM"))
        psS = ctx.enter_context(tc.tile_pool(name="psS", bufs=2, space="PSUM"))
        psT = ctx.enter_context(tc.tile_pool(name="psT", bufs=2, space="PSUM"))
        psO = ctx.enter_context(tc.tile_pool(name="psO", bufs=2, space="PSUM"))
        dpool = ctx.enter_context(tc.tile_pool(name="dram", bufs=2, space="DRAM"))

        # ---------------- constants
        ident = consts.tile([128, 128], BF16, tag="ident")
        make_identity(nc, ident)
        ones32 = consts.tile([32, 1], F32, tag="ones32")
        nc.vector.memset(ones32, 1.0 / 32.0)
        epst = consts.tile([128, 1], F32, tag="epst")
        nc.vector.memset(epst, EPS)
        mask_sb = consts.tile([128, 2, 256], F32, tag="mask_sb")
        nc.sync.dma_start(out=mask_sb, in_=I["masks"].rearrange("m p k -> p m k"))
        cones_sb = consts.tile([128, 8, 4], F32, tag="cones_sb")
        nc.sync.dma_start(out=cones_sb, in_=I["cones"].rearrange("t p c -> p t c"))
        bias_sb = {}
        for i in range(NBLK):
            for nm in (f"b_qkv{i}", f"b_ff1{i}"):
                t = consts.tile([128, I[nm].shape[0]], F32, tag=nm)
                nc.sync.dma_start(out=t, in_=I[nm].rearrange("f p -> p f"))
                bias_sb[nm] = t

        xbuf = pers.tile([128, NT, D], F32, tag="xbuf")

        # ---------------- helpers
        def ln_stats(x_t):
            st = stat.tile([128, 2, nc.vector.BN_STATS_DIM], F32, tag="bnst")
            for i in range(2):
                nc.vector.bn_stats(st[:, i], x_t[:, ts(i, 512)])
            mv = stat.tile([128, nc.vector.BN_AGGR_DIM], F32, tag="bnmv")
            nc.vector.bn_aggr(mv, st)
            sd = stat.tile([128, 1], F32, tag="sd")
            nc.scalar.activation(sd, mv[:, 1:2], AF.Sqrt, bias=epst, scale=1.0)
            rstd = stat.tile([128, 1], F32, tag="rstd")
            nc.vector.reciprocal(rstd, sd)
            nmr = stat.tile([128, 1], F32, tag="nmr")
            nc.vector.scalar_tensor_tensor(nmr, in0=mv[:, 0:1], scalar=-1.0,
                                           in1=rstd, op0=ALU.mult, op1=ALU.mult)
            return rstd, nmr

        def transp(dst, src):
            """dst [f, p] <- src [p, f].T, both bf16 SBUF."""
            p, f = src.shape
            pt = psT.tile([f, p], BF16, tag="pt")
            nc.tensor.transpose(pt, src, ident[:p, :p])
            nc.vector.tensor_copy(dst, pt)

        def rearr_w(name):
            return I[name].rearrange("(k p) f -> p k f", p=128)

        # ---------------- embedding
        with tc.tile_pool(name="emb", bufs=1) as ep:
            oht = ep.tile([128, 2, TALL], BF16, tag="oht")
            nc.sync.dma_start(out=oht, in_=I["oh"].rearrange("(j p) t -> p j t", p=128))
            ehi = ep.tile([128, 2, D], BF16, tag="ehi")
            nc.sync.dma_start(out=ehi, in_=I["emb_hi"].rearrange("(j p) f -> p j f", p=128))
            elo = ep.tile([128, 2, D], BF16, tag="elo")
            nc.sync.dma_start(out=elo, in_=I["emb_lo"].rearrange("(j p) f -> p j f", p=128))
            for t in range(NT):
                for n in range(2):
                    ps = psA.tile([128, 512], F32, tag="ps")
                    for j in range(2):
                        nc.tensor.matmul(ps, oht[:, j, ts(t, 128)], ehi[:, j, ts(n, 512)],
                                         start=(j == 0), stop=False)
                    for j in range(2):
                        nc.tensor.matmul(ps, oht[:, j, ts(t, 128)], elo[:, j, ts(n, 512)],
                                         start=False, stop=(j == 1))
                    pt = ep.tile([128, 512], F32, tag="posx", bufs=3)
                    nc.sync.dma_start(out=pt, in_=I["pos"][ts(t, 128), ts(n, 512)])
                    nc.vector.tensor_add(xbuf[:, t, ts(n, 512)], ps, pt)

        # ---------------- one SWA block
        def swa_block(bi):
            wq = wbig.tile([128, 8, 3 * D], BF16, tag="wbig", name=f"wq{bi}")
            nc.sync.dma_start(out=wq, in_=rearr_w(f"w_qkv{bi}"))
            bq = bias_sb[f"b_qkv{bi}"]

            with tc.tile_pool(name=f"attn{bi}", bufs=1) as pa, \
                 tc.tile_pool(name=f"qk{bi}", bufs=2) as qp:
                xhT = pa.tile([128, 8, TALL], BF16, tag="xhT")
                for t in range(NT):
                    rstd, nmr = ln_stats(xbuf[:, t])
                    xh = work.tile([128, D], BF16, tag="xh")
                    nc.scalar.activation(xh, xbuf[:, t], AF.Identity,
                                         bias=nmr, scale=rstd)
                    for dd in range(8):
                        transp(xhT[:, dd, ts(t, 128)], xh[:, ts(dd, 128)])

                ofm = pa.tile([128, 8, D], BF16, tag="ofm")
                for h in range(NH):
                    qh = qp.tile([128, 2, TRES], BF16, tag="qh")
                    kh = qp.tile([128, 2, TALL], BF16, tag="kh")
                    vh = qp.tile([128, NT, DH], BF16, tag="vh")
                    for dsl in range(2):
                        fc = h * DH + dsl * 128
                        for n in range(2):
                            ps = psA.tile([128, 512], F32, tag="ps")
                            for kk in range(8):
                                nc.tensor.matmul(ps, wq[:, kk, fc:fc + 128],
                                                 xhT[:, kk, 128 + n * 512:128 + (n + 1) * 512],
                                                 start=(kk == 0), stop=(kk == 7))
                            nc.scalar.activation(qh[:, dsl, ts(n, 512)], ps,
                                                 AF.Identity,
                                                 bias=bq[:, 2 * h + dsl:2 * h + dsl + 1])
                        for n in range(3):
                            ps = psA.tile([128, 384], F32, tag="ps")
                            for kk in range(8):
                                nc.tensor.matmul(ps, wq[:, kk, D + fc:D + fc + 128],
                                                 xhT[:, kk, n * 384:(n + 1) * 384],
                                                 start=(kk == 0), stop=(kk == 7))
                            nc.scalar.activation(kh[:, dsl, ts(n, 384)], ps,
                                                 AF.Identity,
                                                 bias=bq[:, 8 + 2 * h + dsl:8 + 2 * h + dsl + 1])
                    vc = 2 * D + h * DH
                    for t in range(NT):
                        ps = psA.tile([128, DH], F32, tag="ps")
                        for kk in range(8):
                            nc.tensor.matmul(ps, xhT[:, kk, ts(t, 128)],
                                             wq[:, kk, vc:vc + DH],
                                             start=(kk == 0), stop=(kk == 7))
                        nc.scalar.copy(vh[:, t], ps)

                    for qb in range(8):
                        sps = psS.tile([128, 256], F32, tag="sps")
                        for dsl in range(2):
                            nc.tensor.matmul(sps, qh[:, dsl, ts(qb, 128)],
                                             kh[:, dsl, qb * 128:qb * 128 + 256],
                                             start=(dsl == 0), stop=(dsl == 1))
                        s_sb = work.tile([128, 256], F32, tag="s_sb")
                        nc.vector.scalar_tensor_tensor(
                            s_sb, in0=sps, scalar=0.0625,
                            in1=mask_sb[:, 0 if qb == 0 else 1],
                            op0=ALU.mult, op1=ALU.add)
                        nmx = stat.tile([128, 1], F32, tag="nmx")
                        nc.vector.tensor_reduce(nmx, s_sb, axis=AX.X,
                                                op=ALU.max, negate=True)
                        p_sb = work.tile([128, 256], F32, tag="p_sb")
                        ssum = stat.tile([128, 1], F32, tag="ssum")
                        nc.scalar.activation(p_sb, s_sb, AF.Exp, bias=nmx,
                                             scale=1.0, accum_out=ssum)
                        rs = stat.tile([128, 1], F32, tag="rs")
                        nc.vector.reciprocal(rs, ssum)
                        pn = work.tile([128, 256], BF16, tag="pn")
                        nc.vector.tensor_scalar_mul(pn, p_sb, rs)
                        pT = work.tile([128, 2, 128], BF16, tag="pT")
                        for kt in range(2):
                            transp(pT[:, kt], pn[:, ts(kt, 128)])
                        for dsl in range(2):
                            po = psO.tile([128, 128], F32, tag="po")
                            for kt in range(2):
                                nc.tensor.matmul(po, vh[:, qb + kt, ts(dsl, 128)],
                                                 pT[:, kt],
                                                 start=(kt == 0), stop=(kt == 1))
                            nc.vector.tensor_copy(ofm[:, h * 2 + dsl, ts(qb, 128)], po)

                # out projection + residual (resident tiles only)
                wout = wsmall.tile([128, 8, D], BF16, tag="wsmall", name=f"wout{bi}")
                nc.sync.dma_start(out=wout, in_=rearr_w(f"w_out{bi}"))
                bbo = bbp.tile([128, D], F32, tag="bbo", name=f"bbo{bi}")
                nc.sync.dma_start(out=bbo, in_=_bcast_row(I[f"b_out{bi}"], 128))
                for t in range(8):
                    for n in range(2):
                        ps = psA.tile([128, 512], F32, tag="ps")
                        for dd in range(8):
                            nc.tensor.matmul(ps, ofm[:, dd, ts(t, 128)],
                                             wout[:, dd, ts(n, 512)],
                                             start=(dd == 0), stop=(dd == 7))
                        nc.vector.tensor_add(ps, ps, bbo[:, ts(n, 512)])
                        nc.vector.tensor_add(xbuf[:, t + 1, ts(n, 512)], ps,
                                             xbuf[:, t + 1, ts(n, 512)])

            # FF
            with tc.tile_pool(name=f"ff{bi}", bufs=1) as pf:
                xh2T = pf.tile([128, 8, TRES], BF16, tag="xh2T")
                for t in range(1, NT):
                    rstd, nmr = ln_stats(xbuf[:, t])
                    xh = work.tile([128, D], BF16, tag="xh")
                    nc.scalar.activation(xh, xbuf[:, t], AF.Identity,
                                         bias=nmr, scale=rstd)
                    for dd in range(8):
                        transp(xh2T[:, dd, ts(t - 1, 128)], xh[:, ts(dd, 128)])

                wf1 = wbig.tile([128, 8, 3 * D], BF16, tag="wbig", name=f"wf1{bi}")
                nc.sync.dma_start(out=wf1, in_=rearr_w(f"w_ff1{bi}"))
                b1 = bias_sb[f"b_ff1{bi}"]
                g1 = pf.tile([128, 24, TRES], BF16, tag="g1")
                for f in range(24):
                    for n in range(2):
                        ps = psA.tile([128, 512], F32, tag="ps")
                        for kk in range(8):
                            nc.tensor.matmul(ps, wf1[:, kk, ts(f, 128)],
                                             xh2T[:, kk, ts(n, 512)],
                                             start=(kk == 0), stop=(kk == 7))
                        nc.scalar.activation(g1[:, f, ts(n, 512)], ps, AF.Gelu,
                                             bias=b1[:, f:f + 1])

                wf2 = wbig.tile([128, 24, D], BF16, tag="wbig", name=f"wf2{bi}")
                nc.sync.dma_start(out=wf2, in_=rearr_w(f"w_ff2{bi}"))
                bb2 = bbp.tile([128, D], F32, tag="bbo", name=f"bb2{bi}")
                nc.sync.dma_start(out=bb2, in_=_bcast_row(I[f"b_ff2{bi}"], 128))
                for t in range(8):
                    for n in range(2):
                        ps = psA.tile([128, 512], F32, tag="ps")
                        for kk in range(24):
                            nc.tensor.matmul(ps, g1[:, kk, ts(t, 128)],
                                             wf2[:, kk, ts(n, 512)],
                                             start=(kk == 0), stop=(kk == 23))
                        nc.vector.tensor_add(ps, ps, bb2[:, ts(n, 512)])
                        nc.vector.tensor_add(xbuf[:, t + 1, ts(n, 512)], ps,
                                             xbuf[:, t + 1, ts(n, 512)])

        def halo_ag(gen):
            hin = dpool.tile([128, D], F32, tag="hin", name=f"hin{gen}")
            hout = dpool.tile([256, D], F32, tag="hout", name=f"hout{gen}")
            nc.sync.dma_start(out=hin, in_=xbuf[:, 8, :])
            nc.gpsimd.collective_compute(
                "AllGather", ALU.bypass,
                replica_groups=[[0, 1], [2, 3], [4, 5], [6, 7]],
                ins=[hin.opt()], outs=[hout.opt()])
            nc.sync.dma_start(out=xbuf[:, 0, :], in_=hout[0:128, :])

        # ---------------- memory stage
        def memory_stage():
            with tc.tile_pool(name="mem", bufs=1) as mp, \
                 tc.tile_pool(name="memw", bufs=1) as mw:
                gline = mp.tile([128, D], BF16, tag="gline")
                nc.gpsimd.dma_start(out=gline, in_=_bcast_row(I["mln_g"], 128))
                bline = mp.tile([128, D], BF16, tag="bline")
                nc.gpsimd.dma_start(out=bline, in_=_bcast_row(I["mln_b"], 128))

                cm_ps = [psS.tile([4, 512], F32, tag="sps", name=f"cm_ps{_n}")
                         for _n in range(2)]
                for t in range(NT):
                    rstd, nmr = ln_stats(xbuf[:, t])
                    hn = mw.tile([128, D], F32, tag="hn", bufs=2)
                    nc.scalar.activation(hn, xbuf[:, t], AF.Identity,
                                         bias=nmr, scale=rstd)
                    nc.vector.tensor_mul(hn, hn, gline)
                    nc.vector.tensor_add(hn, hn, bline)
                    if t >= 1:
                        for n in range(2):
                            nc.tensor.matmul(cm_ps[n], cones_sb[:, t - 1],
                                             hn[:, ts(n, 512)],
                                             start=(t == 1), stop=(t == 8))
                    nc.vector.tensor_sub(xbuf[:, t], xbuf[:, t], hn)
                cm_sb = mp.tile([4, D], F32, tag="cm_sb")
                for n in range(2):
                    nc.vector.tensor_copy(cm_sb[:, ts(n, 512)], cm_ps[n])
                cin = dpool.tile([4, D], F32, tag="cin")
                cout = dpool.tile([32, D], F32, tag="cout")
                nc.sync.dma_start(out=cin, in_=cm_sb)
                nc.gpsimd.collective_compute(
                    "AllGather", ALU.bypass,
                    replica_groups=[list(range(NCORES))],
                    ins=[cin.opt()], outs=[cout.opt()])
                cm32 = mp.tile([32, D], F32, tag="cm32")
                nc.sync.dma_start(out=cm32, in_=cout)

                tgt = mp.tile([32, D], F32, tag="tgt")
                nc.sync.dma_start(out=tgt[0:31], in_=cm32[1:32])
                for b in range(4):
                    nc.sync.dma_start(out=tgt[8 * b + 7:8 * b + 8],
                                      in_=cm32[8 * b + 7:8 * b + 8])

                cmb = mp.tile([32, D], BF16, tag="cmb")
                nc.vector.tensor_copy(cmb, cm32)
                cmT = mp.tile([128, 8, 32], BF16, tag="cmT")
                for dd in range(8):
                    transp(cmT[:, dd], cmb[:, ts(dd, 128)])

                wq_m = mp.tile([128, MH, 8, DK], BF16, tag="wq_m")
                nc.sync.dma_start(out=wq_m, in_=I["mq_w"].rearrange(
                    "h (k p) f -> p h k f", p=128))
                mS_sb = mp.tile([DK, MH, D], BF16, tag="mS_sb")
                nc.sync.dma_start(out=mS_sb, in_=I["mS"].rearrange(
                    "k (h f) -> k h f", h=MH))
                wprec = wbig.tile([128, 8, MH * D], BF16, tag="wbig", name="wprec")
                nc.sync.dma_start(out=wprec, in_=rearr_w("mprec_w"))

                ret3 = mp.tile([32, MH, D], F32, tag="ret3")
                Facc = mp.tile([32, MH], F32, tag="Facc")
                for hh in range(MH):
                    qps = psA.tile([32, DK], F32, tag="ps", name=f"qps{hh}")
                    for dd in range(8):
                        nc.tensor.matmul(qps, cmT[:, dd], wq_m[:, hh, dd],
                                         start=(dd == 0), stop=(dd == 7))
                    bqm = mw.tile([32, DK], F32, tag="bb32", name=f"bqm{hh}")
                    nc.sync.dma_start(out=bqm, in_=_bcast_row(
                        I["mq_b"][:, hh * DK:(hh + 1) * DK], 32))
                    q0 = mw.tile([32, DK], F32, tag="q0")
                    nc.vector.tensor_add(q0, qps, bqm)
                    qsq = mw.tile([32, DK], F32, tag="qsq")
                    nc.vector.tensor_mul(qsq, q0, q0)
                    nrm = stat.tile([32, 1], F32, tag="nrm")
                    nc.vector.tensor_reduce(nrm, qsq, axis=AX.X, op=ALU.add)
                    nc.scalar.sqrt(nrm, nrm)
                    nc.vector.tensor_scalar_max(nrm, nrm, 1e-12)
                    rn = stat.tile([32, 1], F32, tag="rn")
                    nc.vector.reciprocal(rn, nrm)
                    qn = mw.tile([32, DK], BF16, tag="qn")
                    nc.vector.tensor_scalar_mul(qn, q0, rn)
                    qnT = mw.tile([DK, 32], BF16, tag="qnT")
                    transp(qnT, qn)

                    for n in range(2):
                        rps = psA.tile([32, 512], F32, tag="ps", name=f"rps{hh}{n}")
                        nc.tensor.matmul(rps, qnT, mS_sb[:, hh, ts(n, 512)],
                                         start=True, stop=True)
                        nc.vector.tensor_copy(ret3[:, hh, ts(n, 512)], rps)

                    prec = mw.tile([32, D], F32, tag="prec")
                    for n in range(2):
                        zps = psA.tile([32, 512], F32, tag="ps", name=f"zps{hh}{n}")
                        for dd in range(8):
                            nc.tensor.matmul(zps, cmT[:, dd],
                                             wprec[:, dd, hh * D + n * 512:
                                                   hh * D + (n + 1) * 512],
                                             start=(dd == 0), stop=(dd == 7))
                        bpz = mw.tile([32, 512], F32, tag="bb32",
                                      name=f"bpz{hh}{n}")
                        nc.sync.dma_start(out=bpz, in_=_bcast_row(
                            I["mprec_b"][:, hh * D + n * 512:hh * D + (n + 1) * 512], 32))
                        nc.vector.tensor_add(zps, zps, bpz)
                        # softplus(z) = ln(1 + exp(z)); z is O(1) so no overflow
                        ez = mw.tile([32, 512], F32, tag="bb32", name=f"ez{hh}{n}")
                        nc.scalar.activation(ez, zps, AF.Exp)
                        nc.scalar.activation(prec[:, ts(n, 512)], ez, AF.Ln,
                                             bias=1.0)
                    nc.vector.tensor_scalar_add(prec, prec, 0.01)
                    err = mw.tile([32, D], F32, tag="err")
                    nc.vector.tensor_sub(err, tgt, ret3[:, hh])
                    nc.vector.tensor_mul(err, err, err)
                    nc.vector.tensor_mul(err, err, prec)
                    nc.scalar.activation(prec, prec, AF.Ln)
                    junk = mw.tile([32, D], BF16, tag="junk")
                    nc.vector.scalar_tensor_tensor(
                        junk, in0=err, scalar=1.0, in1=prec,
                        op0=ALU.mult, op1=ALU.subtract,
                        accum_out=Facc[:, hh:hh + 1])

                Fm = mp.tile([32, MH], F32, tag="Fm")
                nc.vector.tensor_scalar_mul(Fm, Facc, 1.0 / D)
                mF = stat.tile([32, 1], F32, tag="mF")
                nc.vector.tensor_reduce(mF, Fm, axis=AX.X, op=ALU.min)
                ew = mp.tile([32, MH], F32, tag="ew")
                sw = stat.tile([32, 1], F32, tag="sw")
                nc.scalar.activation(ew, Fm, AF.Exp, bias=mF, scale=-1.0,
                                     accum_out=sw)
                rw = stat.tile([32, 1], F32, tag="rw")
                nc.vector.reciprocal(rw, sw)
                wn = mp.tile([32, MH], F32, tag="wn")
                nc.vector.tensor_scalar_mul(wn, ew, rw)

                moa = mp.tile([32, D], F32, tag="moa")
                nc.vector.memset(moa, 0.0)
                for hh in range(MH):
                    nc.vector.scalar_tensor_tensor(
                        moa, in0=ret3[:, hh], scalar=wn[:, hh:hh + 1], in1=moa,
                        op0=ALU.mult, op1=ALU.add)

                # fe = mean(min_h F)
                feps = psO.tile([1, 1], F32, tag="po", name="feps")
                nc.tensor.matmul(feps, mF, ones32, start=True, stop=True)
                fesb = stat.tile([1, 1], F32, tag="fesb")
                nc.vector.tensor_copy(fesb, feps)
                nc.sync.dma_start(out=fe_ap, in_=fesb)

                wgo = wbig.tile([128, 24, D], BF16, tag="wbig", name="wgo")
                nc.sync.dma_start(out=wgo, in_=rearr_w("m_gw_ow"))
                mob = mp.tile([32, D], BF16, tag="mob")
                nc.vector.tensor_copy(mob, moa)
                moT = mp.tile([128, 8, 32], BF16, tag="moT")
                for dd in range(8):
                    transp(moT[:, dd], mob[:, ts(dd, 128)])
                mo1 = mp.tile([32, D], F32, tag="mo1")
                for n in range(2):
                    ps = psA.tile([32, 512], F32, tag="ps", name=f"mo1ps{n}")
                    for dd in range(8):
                        nc.tensor.matmul(ps, moT[:, dd],
                                         wgo[:, 16 + dd, ts(n, 512)],
                                         start=(dd == 0), stop=(dd == 7))
                    bmo = mw.tile([32, 512], F32, tag="bb32", name=f"bmo{n}")
                    nc.sync.dma_start(out=bmo, in_=_bcast_row(
                        I["moutp_b"][:, ts(n, 512)], 32))
                    nc.vector.tensor_add(mo1[:, ts(n, 512)], ps, bmo)
                mo1b = mp.tile([32, D], BF16, tag="mo1b")
                nc.vector.tensor_copy(mo1b, mo1)
                moT2 = mp.tile([128, 8, 32], BF16, tag="moT2")
                for dd in range(8):
                    transp(moT2[:, dd], mo1b[:, ts(dd, 128)])
                sg = mp.tile([32, D], F32, tag="sg")
                for n in range(2):
                    ps = psA.tile([32, 512], F32, tag="ps", name=f"sgps{n}")
                    for dd in range(8):
                        nc.tensor.matmul(ps, cmT[:, dd], wgo[:, dd, ts(n, 512)],
                                         start=(dd == 0), stop=False)
                    for dd in range(8):
                        nc.tensor.matmul(ps, moT2[:, dd],
                                         wgo[:, 8 + dd, ts(n, 512)],
                                         start=False, stop=(dd == 7))
                    bga = mw.tile([32, 512], F32, tag="bb32", name=f"bga{n}")
                    nc.sync.dma_start(out=bga, in_=_bcast_row(
                        I["mgate_b"][:, ts(n, 512)], 32))
                    nc.vector.tensor_add(ps, ps, bga)
                    nc.scalar.activation(sg[:, ts(n, 512)], ps, AF.Sigmoid)
                dd_t = mw.tile([32, D], F32, tag="err", name="dd_t")
                nc.vector.tensor_sub(dd_t, mo1, cm32)
                tt_t = mw.tile([32, D], F32, tag="prec", name="tt_t")
                nc.vector.scalar_tensor_tensor(tt_t, in0=sg, scalar=0.3,
                                               in1=dd_t, op0=ALU.mult,
                                               op1=ALU.mult)
                mo_f = mp.tile([32, D], F32, tag="mo_f")
                nc.vector.tensor_add(mo_f, cm32, tt_t)

                sel_sb = mp.tile([32, NT, 128], F32, tag="sel_sb")
                nc.sync.dma_start(out=sel_sb, in_=I["sel"].rearrange("t k m -> k t m"))
                for t in range(NT):
                    for n in range(2):
                        ps = psA.tile([128, 512], F32, tag="ps",
                                      name=f"selps{t}{n}")
                        nc.tensor.matmul(ps, sel_sb[:, t], mo_f[:, ts(n, 512)],
                                         start=True, stop=True)
                        nc.vector.tensor_add(xbuf[:, t, ts(n, 512)], ps,
                                             xbuf[:, t, ts(n, 512)])

        # ---------------- schedule
        swa_block(0)
        halo_ag(0)
        swa_block(1)
        halo_ag(1)
        memory_stage()
        swa_block(2)
        halo_ag(2)
        swa_block(3)

        # ---------------- head
        with tc.tile_pool(name="headp", bufs=2) as hp:
            whd = wsmall.tile([128, 8, 256], BF16, tag="wsmall", name="whd")
            nc.sync.dma_start(out=whd, in_=rearr_w("head_w"))
            hbb = bbp.tile([128, 256], F32, tag="bbo", name="hbb")
            nc.sync.dma_start(out=hbb, in_=_bcast_row(I["head_b"], 128))
            for t in range(1, NT):
                rstd, nmr = ln_stats(xbuf[:, t])
                xh = work.tile([128, D], BF16, tag="xh")
                nc.scalar.activation(xh, xbuf[:, t], AF.Identity,
                                     bias=nmr, scale=rstd)
                xhTt = hp.tile([128, 8, 128], BF16, tag="xhTt")
                for dd in range(8):
                    transp(xhTt[:, dd], xh[:, ts(dd, 128)])
                ps = psA.tile([128, 256], F32, tag="ps")
                for dd in range(8):
                    nc.tensor.matmul(ps, xhTt[:, dd], whd[:, dd],
                                     start=(dd == 0), stop=(dd == 7))
                lg = work.tile([128, 256], F32, tag="lg")
                nc.vector.tensor_add(lg, ps, hbb)
                nc.sync.dma_start(out=logits_ap[ts(t - 1, 128), :], in_=lg)


# ---------------------------------------------------------------- host side
def _np32(x):
    return np.asarray(x, np.float32)


def _fold_ln(w, b_lin, g, b_ln):
    wg = _np32(w) * _np32(g)[:, None]
    bb = _np32(b_ln) @ _np32(w) + _np32(b_lin)
    return wg.astype(NPBF16), bb.astype(np.float32)


def _make_masks(is_even):
    x = np.arange(128)[:, None]
    y = np.arange(256)[None, :]
    band = (y >= x + 1) & (y <= x + 128)
    first = band & (y >= 128) if is_even else band
    m = np.zeros((2, 128, 256), np.float32)
    m[0][~first] = -1e30
    m[1][~band] = -1e30
    return m


def _prep_inputs(byte_ids, params):
    byte_ids = np.asarray(byte_ids)
    p = params
    emb = _np32(p["byte_emb"])
    ehi = emb.astype(NPBF16)
    elo = (emb - ehi.astype(np.float32)).astype(NPBF16)
    pos = _np32(p["pos_emb"])

    shared = {"emb_hi": ehi, "emb_lo": elo}
    blocks = list(p["lower"]) + list(p["upper"])
    for i, bp in enumerate(blocks):
        wq, bq = _fold_ln(bp["qkv_w"], bp["qkv_b"], bp["ln1_g"], bp["ln1_b"])
        w1, b1 = _fold_ln(bp["ff1_w"], bp["ff1_b"], bp["ln2_g"], bp["ln2_b"])
        b_v = bq[2 * D:]
        b_out = _np32(bp["out_b"]) + b_v @ _np32(bp["out_w"])
        shared[f"w_qkv{i}"] = wq
        shared[f"b_qkv{i}"] = bq[:2 * D].reshape(16, 128)
        shared[f"w_out{i}"] = _np32(bp["out_w"]).astype(NPBF16)
        shared[f"b_out{i}"] = b_out[None, :].astype(np.float32)
        shared[f"w_ff1{i}"] = w1
        shared[f"b_ff1{i}"] = b1.reshape(24, 128)
        shared[f"w_ff2{i}"] = _np32(bp["ff2_w"]).astype(NPBF16)
        shared[f"b_ff2{i}"] = _np32(bp["ff2_b"])[None, :].astype(np.float32)
    m = p["mem"]
    shared["mln_g"] = _np32(m["ln_g"])[None, :]
    shared["mln_b"] = _np32(m["ln_b"])[None, :]
    shared["mq_w"] = np.stack([_np32(h["q_w"]) for h in m["heads"]]).astype(NPBF16)
    shared["mq_b"] = np.concatenate(
        [_np32(h["q_b"]) for h in m["heads"]])[None, :].astype(np.float32)
    shared["mprec_w"] = np.concatenate(
        [_np32(h["prec_w"]) for h in m["heads"]], axis=1).astype(NPBF16)
    shared["mprec_b"] = np.concatenate(
        [_np32(h["prec_b"]) for h in m["heads"]])[None, :].astype(np.float32)
    shared["mS"] = np.concatenate(
        [_np32(h["S"]) for h in m["heads"]], axis=1).astype(NPBF16)
    shared["m_gw_ow"] = np.concatenate(
        [_np32(m["gate_w"]), _np32(m["outp_w"])], axis=0).astype(NPBF16)
    shared["mgate_b"] = _np32(m["gate_b"])[None, :]
    shared["moutp_b"] = _np32(m["outp_b"])[None, :]
    hw, hb = _fold_ln(p["head_w"], p["head_b"], p["lno_g"], p["lno_b"])
    shared["head_w"] = hw
    shared["head_b"] = hb[None, :]

    cones = np.zeros((8, 128, 4), np.float32)
    for t in range(8):
        cones[t, :, t // 2] = 1.0 / CS

    in_maps = []
    for c in range(NCORES):
        b, half = c // 2, c % 2
        s0 = half * TRES
        ids_pad = np.full(TALL, -1, np.int64)
        lo = max(0, s0 - 128)
        ids_pad[128 - (s0 - lo):] = byte_ids[b, lo:s0 + TRES]
        oh = np.zeros((256, TALL), np.float32)
        valid = ids_pad >= 0
        oh[ids_pad[valid], np.nonzero(valid)[0]] = 1.0
        posc = np.zeros((TALL, D), np.float32)
        posc[128 - (s0 - lo):] = pos[lo:s0 + TRES]
        sel = np.zeros((NT, 32, 128), np.float32)
        for t in range(NT):
            g = c * 4 + (t - 1) // 2
            if t == 0:
                g = c * 4 - 1 if half == 1 else 0
            sel[t, g, :] = 1.0
        m_ = {"oh": oh.astype(NPBF16), "pos": posc,
              "masks": _make_masks(half == 0), "sel": sel, "cones": cones}
        m_.update(shared)
        in_maps.append(m_)
    return in_maps


def kernel(byte_ids, params):
    if "nc" not in _CACHE:
        _CACHE["nc"] = _build()
    nc = _CACHE["nc"]
    in_maps = _prep_inputs(byte_ids, params)
    res = bass_utils.run_bass_kernel_spmd(nc, in_maps,
                                          core_ids=list(range(NCORES)))
    logits = np.zeros((B, S, 256), np.float32)
    for c in range(NCORES):
        b, half = c // 2, c % 2
        logits[b, half * TRES:(half + 1) * TRES] = res.results[c]["logits"]
    fe = np.float32(res.results[0]["fe"][0, 0])
    return logits, fe


# revision 31
# speedup vs baseline: 13322.5711x; 13322.5711x over previous
"""Trainium2 Bass kernel for nn_FEPModel (byte-LM with sliding-window attention
blocks around a delta-rule chunk-memory module).

Sharding: 8 cores = 4 batches x 2 sequence halves. Each core holds 1152 tokens
(128-token left halo + 1024 resident), runs the full model on its shard.
Cross-core traffic: 3 pairwise halo AllGathers (0.5 MB) between SWA blocks and
one 8-way AllGather of the 4 chunk means (16 KB) for the memory stage.

Layouts: residual stream token-major [128, 9, 1024] f32 in SBUF. LayerNorm via
bn_stats (free-dim reduce). LN gain/bias folded into the following linear's
weights on host; v-projection bias folded into out-proj bias (softmax rows sum
to 1). Matmuls in bf16 with f32 PSUM accumulation. Feature-major linear biases
are fused into PSUM-eviction activations as per-partition [128,1] operands;
token-major biases are added from DMA-broadcast rows.
"""

import numpy as np

try:
    import concourse.bass as bass
except ImportError:
    import sys
    sys.path.insert(0, "/opt/trn_rl_repo")
    import concourse.bass as bass

import ml_dtypes
import concourse.bacc as bacc
import concourse.mybir as mybir
import concourse.tile as tile
import concourse.bass_utils as bass_utils
from concourse.bass import ts
from concourse.masks import make_identity

F32 = mybir.dt.float32
BF16 = mybir.dt.bfloat16
NPBF16 = ml_dtypes.bfloat16
AF = mybir.ActivationFunctionType
ALU = mybir.AluOpType
AX = mybir.AxisListType

B, S, D = 4, 2048, 1024
NH, DH = 4, D // 4
CS, DK, MH = 256, 64, 3
EPS = 1e-5
NCORES = 8
TRES, TALL, NT = 1024, 1152, 9
NBLK = 4

_CACHE = {}


def _bcast_row(ap, parts):
    """DRAM [1, N] row -> partition-broadcast AP [parts, N]."""
    return bass.AP(tensor=ap.tensor, offset=ap.offset,
                   ap=[[0, parts]] + list(ap.ap[1:]))


# ---------------------------------------------------------------- device build
def _declare_inputs(nc):
    d = {}

    def di(name, shape, dt):
        d[name] = nc.dram_tensor(name, list(shape), dt, kind="ExternalInput").ap()

    di("emb_hi", (256, D), BF16)
    di("emb_lo", (256, D), BF16)
    di("oh", (256, TALL), BF16)
    di("pos", (TALL, D), F32)
    di("masks", (2, 128, 256), F32)
    di("sel", (NT, 32, 128), F32)
    di("cones", (8, 128, 4), F32)
    for i in range(NBLK):
        di(f"w_qkv{i}", (D, 3 * D), BF16)
        di(f"b_qkv{i}", (16, 128), F32)     # q,k bias, partition-major
        di(f"w_out{i}", (D, D), BF16)
        di(f"b_out{i}", (1, D), F32)        # includes folded v bias
        di(f"w_ff1{i}", (D, 3 * D), BF16)
        di(f"b_ff1{i}", (24, 128), F32)     # partition-major
        di(f"w_ff2{i}", (3 * D, D), BF16)
        di(f"b_ff2{i}", (1, D), F32)
    di("mln_g", (1, D), F32)
    di("mln_b", (1, D), F32)
    di("mq_w", (MH, D, DK), BF16)
    di("mq_b", (1, MH * DK), F32)
    di("mprec_w", (D, MH * D), BF16)
    di("mprec_b", (1, MH * D), F32)
    di("mS", (DK, MH * D), BF16)
    di("m_gw_ow", (3 * D, D), BF16)         # rows 0:2048 gate_w, 2048:3072 outp_w
    di("mgate_b", (1, D), F32)
    di("moutp_b", (1, D), F32)
    di("head_w", (D, 256), BF16)
    di("head_b", (1, 256), F32)
    return d


def _build():
    nc = bacc.Bacc("TRN2", target_bir_lowering=False, debug=False,
                   enable_asserts=False, num_devices=NCORES)
    I = _declare_inputs(nc)
    logits_ap = nc.dram_tensor("logits", [TRES, 256], F32,
                               kind="ExternalOutput").ap()
    fe_ap = nc.dram_tensor("fe", [1, 1], F32, kind="ExternalOutput").ap()

    with tile.TileContext(nc) as tc:
        _program(nc, tc, I, logits_ap, fe_ap)
    nc.compile()
    return nc


def _program(nc, tc, I, logits_ap, fe_ap):
    import contextlib
    ctx = contextlib.ExitStack()
    with ctx:
        pers = ctx.enter_context(tc.tile_pool(name="pers", bufs=1))
        consts = ctx.enter_context(tc.tile_pool(name="consts", bufs=1))
        wbig = ctx.enter_context(tc.tile_pool(name="wbig", bufs=1))
        wsmall = ctx.enter_context(tc.tile_pool(name="wsmall", bufs=1))
        work = ctx.enter_context(tc.tile_pool(name="work", bufs=2))
        stat = ctx.enter_context(tc.tile_pool(name="stat", bufs=4))
        bbp = ctx.enter_context(tc.tile_pool(name="bbp", bufs=1))
        psA = ctx.enter_context(tc.tile_pool(name="psA", bufs=4, space="PSUM"))
        psS = ctx.enter_context(tc.tile_pool(name="psS", bufs=2, space="PSUM"))
        psT = ctx.enter_context(tc.tile_pool(name="psT", bufs=1, space="PSUM"))
        psO = ctx.enter_context(tc.tile_pool(name="psO", bufs=1, space="PSUM"))
        dpool = ctx.enter_context(tc.tile_pool(name="dram", bufs=2, space="DRAM"))

        # ---------------- constants
        ident = consts.tile([128, 128], BF16, tag="ident")
        make_identity(nc, ident)
        ones32 = consts.tile([32, 1], F32, tag="ones32")
        nc.vector.memset(ones32, 1.0 / 32.0)
        ones1 = consts.tile([1, 128], F32, tag="ones1")
        nc.vector.memset(ones1, 1.0)
        epst = consts.tile([128, 1], F32, tag="epst")
        nc.vector.memset(epst, EPS)
        mask_sb = consts.tile([128, 2, 256], F32, tag="mask_sb")
        nc.sync.dma_start(out=mask_sb, in_=I["masks"].rearrange("m p k -> p m k"))
        cones_sb = consts.tile([128, 8, 4], F32, tag="cones_sb")
        nc.sync.dma_start(out=cones_sb, in_=I["cones"].rearrange("t p c -> p t c"))
        bias_sb = {}
        for i in range(NBLK):
            for nm in (f"b_qkv{i}", f"b_ff1{i}"):
                t = consts.tile([128, I[nm].shape[0]], F32, tag=nm)
                nc.sync.dma_start(out=t, in_=I[nm].rearrange("f p -> p f"))
                bias_sb[nm] = t

        xbuf = pers.tile([128, NT, D], F32, tag="xbuf")

        # ---------------- helpers
        def ln_stats(x_t):
            st = stat.tile([128, 2, nc.vector.BN_STATS_DIM], F32, tag="bnst")
            for i in range(2):
                nc.vector.bn_stats(st[:, i], x_t[:, ts(i, 512)])
            mv = stat.tile([128, nc.vector.BN_AGGR_DIM], F32, tag="bnmv")
            nc.vector.bn_aggr(mv, st)
            sd = stat.tile([128, 1], F32, tag="sd")
            nc.scalar.activation(sd, mv[:, 1:2], AF.Ln, bias=epst, scale=1.0)
            rstd = stat.tile([128, 1], F32, tag="rstd")
            nc.scalar.activation(rstd, sd, AF.Exp, scale=-0.5)
            nmr = stat.tile([128, 1], F32, tag="nmr")
            nc.vector.scalar_tensor_tensor(nmr, in0=mv[:, 0:1], scalar=-1.0,
                                           in1=rstd, op0=ALU.mult, op1=ALU.mult)
            return rstd, nmr

        def transp(dst, src):
            """dst [f, p] <- src [p, f].T, both bf16 SBUF."""
            p, f = src.shape
            pt = psT.tile([f, p], BF16, tag="pt")
            nc.tensor.transpose(pt, src, ident[:p, :p])
            nc.vector.tensor_copy(dst, pt)

        def rearr_w(name):
            return I[name].rearrange("(k p) f -> p k f", p=128)

        # ---------------- embedding
        with tc.tile_pool(name="emb", bufs=1) as ep:
            oht = ep.tile([128, 2, TALL], BF16, tag="oht")
            nc.sync.dma_start(out=oht, in_=I["oh"].rearrange("(j p) t -> p j t", p=128))
            ehi = ep.tile([128, 2, D], BF16, tag="ehi")
            nc.sync.dma_start(out=ehi, in_=I["emb_hi"].rearrange("(j p) f -> p j f", p=128))
            elo = ep.tile([128, 2, D], BF16, tag="elo")
            nc.sync.dma_start(out=elo, in_=I["emb_lo"].rearrange("(j p) f -> p j f", p=128))
            for t in range(NT - 1, -1, -1):
                for n in range(2):
                    ps = psA.tile([128, 512], F32, tag="ps")
                    for j in range(2):
                        nc.tensor.matmul(ps, oht[:, j, ts(t, 128)], ehi[:, j, ts(n, 512)],
                                         start=(j == 0), stop=False)
                    for j in range(2):
                        nc.tensor.matmul(ps, oht[:, j, ts(t, 128)], elo[:, j, ts(n, 512)],
                                         start=False, stop=(j == 1))
                    pt = ep.tile([128, 512], F32, tag="posx", bufs=3)
                    nc.sync.dma_start(out=pt, in_=I["pos"][ts(t, 128), ts(n, 512)])
                    nc.vector.tensor_add(xbuf[:, t, ts(n, 512)], ps, pt)

        # ---------------- one SWA block
        def swa_block(bi):
            wq = wbig.tile([128, 8, 3 * D], BF16, tag="wbig", name=f"wq{bi}")
            for _c in range(4):
                nc.sync.dma_start(out=wq[:, ts(_c, 2)],
                                  in_=rearr_w(f"w_qkv{bi}")[:, ts(_c, 2)])
            bq = bias_sb[f"b_qkv{bi}"]

            with tc.tile_pool(name=f"attn{bi}", bufs=1) as pa, \
                 tc.tile_pool(name=f"qk{bi}", bufs=2) as qp:
                xhT = pa.tile([128, 8, TALL], BF16, tag="xhT")
                for t in list(range(NT - 1, 0, -1)) + [0]:
                    rstd, nmr = ln_stats(xbuf[:, t])
                    xh = work.tile([128, D], BF16, tag="xh")
                    nc.scalar.activation(xh, xbuf[:, t], AF.Identity,
                                         bias=nmr, scale=rstd)
                    for dd in range(8):
                        transp(xhT[:, dd, ts(t, 128)], xh[:, ts(dd, 128)])

                ofm = pa.tile([128, 8, D], BF16, tag="ofm")
                for h in range(NH):
                    qh = qp.tile([128, 2, TRES], BF16, tag="qh")
                    kh = qp.tile([128, 2, TALL], BF16, tag="kh")
                    vh = qp.tile([128, NT, DH], BF16, tag="vh")
                    for dsl in range(2):
                        fc = h * DH + dsl * 128
                        for n in range(2):
                            ps = psA.tile([128, 512], F32, tag="ps")
                            for kk in range(8):
                                nc.tensor.matmul(ps, wq[:, kk, fc:fc + 128],
                                                 xhT[:, kk, 128 + n * 512:128 + (n + 1) * 512],
                                                 start=(kk == 0), stop=(kk == 7))
                            nc.scalar.activation(qh[:, dsl, ts(n, 512)], ps,
                                                 AF.Identity,
                                                 bias=bq[:, 2 * h + dsl:2 * h + dsl + 1])
                        for n in (1, 2, 0):
                            ps = psA.tile([128, 384], F32, tag="ps")
                            for kk in range(8):
                                nc.tensor.matmul(ps, wq[:, kk, D + fc:D + fc + 128],
                                                 xhT[:, kk, n * 384:(n + 1) * 384],
                                                 start=(kk == 0), stop=(kk == 7))
                            nc.scalar.activation(kh[:, dsl, ts(n, 384)], ps,
                                                 AF.Identity,
                                                 bias=bq[:, 8 + 2 * h + dsl:8 + 2 * h + dsl + 1])
                    vc = 2 * D + h * DH
                    for t in list(range(NT - 1, 0, -1)) + [0]:
                        ps = psA.tile([128, DH], F32, tag="ps")
                        for kk in range(8):
                            nc.tensor.matmul(ps, xhT[:, kk, ts(t, 128)],
                                             wq[:, kk, vc:vc + DH],
                                             start=(kk == 0), stop=(kk == 7))
                        nc.scalar.copy(vh[:, t], ps)

                    for qb in range(7, -1, -1):
                        sps = psS.tile([128, 256], F32, tag="sps")
                        for dsl in range(2):
                            nc.tensor.matmul(sps, qh[:, dsl, ts(qb, 128)],
                                             kh[:, dsl, qb * 128:qb * 128 + 256],
                                             start=(dsl == 0), stop=(dsl == 1))
                        s_sb = work.tile([128, 256], F32, tag="s_sb")
                        nc.vector.tensor_add(s_sb, sps,
                                             mask_sb[:, 0 if qb == 0 else 1])
                        nmx = stat.tile([128, 1], F32, tag="nmx")
                        nc.vector.tensor_reduce(nmx, s_sb, axis=AX.X,
                                                op=ALU.max, negate=True)
                        p_sb = work.tile([128, 256], F32, tag="p_sb")
                        ssum = stat.tile([128, 1], F32, tag="ssum")
                        nc.scalar.activation(p_sb, s_sb, AF.Exp, bias=nmx,
                                             scale=1.0, accum_out=ssum)
                        rs = stat.tile([128, 1], F32, tag="rs")
                        nc.vector.reciprocal(rs, ssum)
                        pn = work.tile([128, 256], BF16, tag="pn")
                        nc.vector.tensor_scalar_mul(pn, p_sb, rs)
                        pT = work.tile([128, 2, 128], BF16, tag="pT")
                        for kt in range(2):
                            transp(pT[:, kt], pn[:, ts(kt, 128)])
                        for dsl in range(2):
                            po = psO.tile([128, 128], F32, tag="po")
                            for kt in range(2):
                                nc.tensor.matmul(po, vh[:, qb + kt, ts(dsl, 128)],
                                                 pT[:, kt],
                                                 start=(kt == 0), stop=(kt == 1))
                            nc.vector.tensor_copy(ofm[:, h * 2 + dsl, ts(qb, 128)], po)

                # out projection + residual (resident tiles only)
                wout = wsmall.tile([128, 8, D], BF16, tag="wsmall", name=f"wout{bi}")
                nc.sync.dma_start(out=wout, in_=rearr_w(f"w_out{bi}"))
                bbo = bbp.tile([128, D], F32, tag="bbo", name=f"bbo{bi}")
                nc.sync.dma_start(out=bbo, in_=_bcast_row(I[f"b_out{bi}"], 128))
                for t in (range(8) if bi == 3 else range(7, -1, -1)):
                    for n in range(2):
                        ps = psA.tile([128, 512], F32, tag="ps")
                        for dd in range(8):
                            nc.tensor.matmul(ps, ofm[:, dd, ts(t, 128)],
                                             wout[:, dd, ts(n, 512)],
                                             start=(dd == 0), stop=False)
                        nc.tensor.matmul(ps, ones1, bbo[0:1, ts(n, 512)],
                                         start=False, stop=True)
                        nc.vector.tensor_add(xbuf[:, t + 1, ts(n, 512)], ps,
                                             xbuf[:, t + 1, ts(n, 512)])

            # FF
            with tc.tile_pool(name=f"ff{bi}", bufs=1) as pf:
                xh2T = pf.tile([128, 8, TRES], BF16, tag="xh2T")
                for t in (range(1, NT) if bi == 3 else range(NT - 1, 0, -1)):
                    rstd, nmr = ln_stats(xbuf[:, t])
                    xh = work.tile([128, D], BF16, tag="xh")
                    nc.scalar.activation(xh, xbuf[:, t], AF.Identity,
                                         bias=nmr, scale=rstd)
                    for dd in range(8):
                        transp(xh2T[:, dd, ts(t - 1, 128)], xh[:, ts(dd, 128)])

                wf1 = wbig.tile([128, 8, 3 * D], BF16, tag="wbig", name=f"wf1{bi}")
                for _c in range(4):
                    nc.sync.dma_start(out=wf1[:, ts(_c, 2)],
                                      in_=rearr_w(f"w_ff1{bi}")[:, ts(_c, 2)])
                b1 = bias_sb[f"b_ff1{bi}"]
                g1 = pf.tile([128, 24, TRES], BF16, tag="g1")
                for f in range(24):
                    for n in ((0, 1) if bi == 3 else (1, 0)):
                        ps = psA.tile([128, 512], F32, tag="ps")
                        for kk in range(8):
                            nc.tensor.matmul(ps, wf1[:, kk, ts(f, 128)],
                                             xh2T[:, kk, ts(n, 512)],
                                             start=(kk == 0), stop=(kk == 7))
                        nc.scalar.activation(g1[:, f, ts(n, 512)], ps, AF.Gelu,
                                             bias=b1[:, f:f + 1])

                wf2 = wbig.tile([128, 24, D], BF16, tag="wbig", name=f"wf2{bi}")
                for _c in range(4):
                    nc.sync.dma_start(out=wf2[:, ts(_c, 6)],
                                      in_=rearr_w(f"w_ff2{bi}")[:, ts(_c, 6)])
                bb2 = bbp.tile([128, D], F32, tag="bbo", name=f"bb2{bi}")
                nc.sync.dma_start(out=bb2, in_=_bcast_row(I[f"b_ff2{bi}"], 128))
                for t in (range(8) if bi == 3 else range(7, -1, -1)):
                    for n in range(2):
                        ps = psA.tile([128, 512], F32, tag="ps")
                        for kk in range(24):
                            nc.tensor.matmul(ps, g1[:, kk, ts(t, 128)],
                                             wf2[:, kk, ts(n, 512)],
                                             start=(kk == 0), stop=False)
                        nc.tensor.matmul(ps, ones1, bb2[0:1, ts(n, 512)],
                                         start=False, stop=True)
                        nc.vector.tensor_add(xbuf[:, t + 1, ts(n, 512)], ps,
                                             xbuf[:, t + 1, ts(n, 512)])

        def halo_ag(gen):
            hin = dpool.tile([128, D], F32, tag="hin", name=f"hin{gen}")
            hout = dpool.tile([256, D], F32, tag="hout", name=f"hout{gen}")
            nc.sync.dma_start(out=hin, in_=xbuf[:, 8, :])
            nc.gpsimd.collective_compute(
                "AllGather", ALU.bypass,
                replica_groups=[[0, 1], [2, 3], [4, 5], [6, 7]],
                ins=[hin.opt()], outs=[hout.opt()])
            nc.sync.dma_start(out=xbuf[:, 0, :], in_=hout[0:128, :])

        # ---------------- memory stage
        def memory_stage():
            with tc.tile_pool(name="mem", bufs=1) as mp, \
                 tc.tile_pool(name="memw", bufs=1) as mw:
                gline = mp.tile([128, D], BF16, tag="gline")
                nc.gpsimd.dma_start(out=gline, in_=_bcast_row(I["mln_g"], 128))
                bline = mp.tile([128, D], BF16, tag="bline")
                nc.gpsimd.dma_start(out=bline, in_=_bcast_row(I["mln_b"], 128))

                # x <- x - (xhat*g + b); cm = chunkmean(xhat)*g + b.
                # The b-part of the x update is folded into mo_f at the end
                # (mo_f' = mo_f - b), the g,b of cm applied post-mean.
                cm_ps = [psS.tile([4, 512], F32, tag="sps", name=f"cm_ps{_n}")
                         for _n in range(2)]
                for t in list(range(NT - 1, 0, -1)) + [0]:
                    rstd, nmr = ln_stats(xbuf[:, t])
                    hn = mw.tile([128, D], F32, tag="hn", bufs=3)
                    nc.scalar.activation(hn, xbuf[:, t], AF.Identity,
                                         bias=nmr, scale=rstd)
                    if t >= 1:
                        for n in range(2):
                            nc.tensor.matmul(cm_ps[n], cones_sb[:, t - 1],
                                             hn[:, ts(n, 512)],
                                             start=(t == 8), stop=(t == 1))
                    nc.vector.tensor_mul(hn, hn, gline)
                    nc.vector.tensor_sub(xbuf[:, t], xbuf[:, t], hn)
                cm_sb = mp.tile([4, D], F32, tag="cm_sb")
                for n in range(2):
                    nc.vector.tensor_copy(cm_sb[:, ts(n, 512)], cm_ps[n])
                nc.vector.tensor_mul(cm_sb, cm_sb, gline[:4])
                nc.vector.tensor_add(cm_sb, cm_sb, bline[:4])
                cin = dpool.tile([4, D], F32, tag="cin")
                cout = dpool.tile([32, D], F32, tag="cout")
                nc.sync.dma_start(out=cin, in_=cm_sb)
                nc.gpsimd.collective_compute(
                    "AllGather", ALU.bypass,
                    replica_groups=[list(range(NCORES))],
                    ins=[cin.opt()], outs=[cout.opt()])
                cm32 = mp.tile([32, D], F32, tag="cm32")
                nc.sync.dma_start(out=cm32, in_=cout)
                import os as _os2
                if _os2.environ.get("KABL_MEM_LITE", "") == "1":
                    return

                tgt = mp.tile([32, D], F32, tag="tgt")
                nc.sync.dma_start(out=tgt[0:31], in_=cm32[1:32])
                for b in range(4):
                    nc.sync.dma_start(out=tgt[8 * b + 7:8 * b + 8],
                                      in_=cm32[8 * b + 7:8 * b + 8])

                cmb = mp.tile([32, D], BF16, tag="cmb")
                nc.vector.tensor_copy(cmb, cm32)
                cmT = mp.tile([128, 8, 32], BF16, tag="cmT")
                for dd in range(8):
                    transp(cmT[:, dd], cmb[:, ts(dd, 128)])





                ret3 = mp.tile([32, MH, D], F32, tag="ret3")
                Facc = mp.tile([32, MH], F32, tag="Facc")
                for hh in range(MH):
                    wq_h = mw.tile([128, 8, DK], BF16, tag="wq_h", bufs=2,
                                   name=f"wq_h{hh}")
                    nc.sync.dma_start(out=wq_h, in_=I["mq_w"][hh].rearrange(
                        "(k p) f -> p k f", p=128))
                    qps = psA.tile([32, DK], F32, tag="ps", name=f"qps{hh}")
                    for dd in range(8):
                        nc.tensor.matmul(qps, cmT[:, dd], wq_h[:, dd],
                                         start=(dd == 0), stop=(dd == 7))
                    bqm = mw.tile([32, DK], F32, tag="bb32", bufs=3, name=f"bqm{hh}")
                    nc.sync.dma_start(out=bqm, in_=_bcast_row(
                        I["mq_b"][:, hh * DK:(hh + 1) * DK], 32))
                    q0 = mw.tile([32, DK], F32, tag="q0", bufs=2)
                    nc.vector.tensor_add(q0, qps, bqm)
                    qsq = mw.tile([32, DK], F32, tag="qsq", bufs=2)
                    nc.vector.tensor_mul(qsq, q0, q0)
                    nrm = stat.tile([32, 1], F32, tag="nrm")
                    nc.vector.tensor_reduce(nrm, qsq, axis=AX.X, op=ALU.add)
                    nc.vector.tensor_scalar_max(nrm, nrm, 1e-24)
                    nc.scalar.activation(nrm, nrm, AF.Ln)
                    rn = stat.tile([32, 1], F32, tag="rn")
                    nc.scalar.activation(rn, nrm, AF.Exp, scale=-0.5)
                    qn = mw.tile([32, DK], BF16, tag="qn", bufs=2)
                    nc.vector.tensor_scalar_mul(qn, q0, rn)
                    qnT = mw.tile([DK, 32], BF16, tag="qnT", bufs=2)
                    transp(qnT, qn)

                    mS_h = mw.tile([DK, D], BF16, tag="mS_h", bufs=2,
                                   name=f"mS_h{hh}")
                    nc.sync.dma_start(out=mS_h,
                                      in_=I["mS"][:, hh * D:(hh + 1) * D])
                    for n in range(2):
                        rps = psA.tile([32, 512], F32, tag="ps", name=f"rps{hh}{n}")
                        nc.tensor.matmul(rps, qnT, mS_h[:, ts(n, 512)],
                                         start=True, stop=True)
                        nc.vector.tensor_copy(ret3[:, hh, ts(n, 512)], rps)

                    wprec = wsmall.tile([128, 8, D], BF16, tag="wsmall",
                                        name=f"wprec{hh}")
                    nc.sync.dma_start(
                        out=wprec,
                        in_=rearr_w("mprec_w")[:, :, hh * D:(hh + 1) * D])
                    prec = mw.tile([32, D], F32, tag="prec", bufs=2)
                    for n in range(2):
                        zps = psA.tile([32, 512], F32, tag="ps", name=f"zps{hh}{n}")
                        for dd in range(8):
                            nc.tensor.matmul(zps, cmT[:, dd],
                                             wprec[:, dd, ts(n, 512)],
                                             start=(dd == 0), stop=(dd == 7))
                        bpz = mw.tile([32, 512], F32, tag="bb32", bufs=3,
                                      name=f"bpz{hh}{n}")
                        nc.sync.dma_start(out=bpz, in_=_bcast_row(
                            I["mprec_b"][:, hh * D + n * 512:hh * D + (n + 1) * 512], 32))
                        nc.vector.tensor_add(zps, zps, bpz)
                        # softplus(z) = ln(1 + exp(z)); z is O(1) so no overflow
                        ez = mw.tile([32, 512], F32, tag="bb32", bufs=3, name=f"ez{hh}{n}")
                        nc.scalar.activation(ez, zps, AF.Exp)
                        nc.scalar.activation(prec[:, ts(n, 512)], ez, AF.Ln,
                                             bias=1.0)
                    nc.vector.tensor_scalar_add(prec, prec, 0.01)
                    err = mw.tile([32, D], F32, tag="err", bufs=2)
                    nc.vector.tensor_sub(err, tgt, ret3[:, hh])
                    nc.vector.tensor_mul(err, err, err)
                    nc.vector.tensor_mul(err, err, prec)
                    nc.scalar.activation(prec, prec, AF.Ln)
                    nc.vector.scalar_tensor_tensor(
                        err, in0=err, scalar=1.0, in1=prec,
                        op0=ALU.mult, op1=ALU.subtract,
                        accum_out=Facc[:, hh:hh + 1])

                Fm = mp.tile([32, MH], F32, tag="Fm")
                nc.vector.tensor_scalar_mul(Fm, Facc, 1.0 / D)
                mF = stat.tile([32, 1], F32, tag="mF")
                nc.vector.tensor_reduce(mF, Fm, axis=AX.X, op=ALU.min)
                ew = mp.tile([32, MH], F32, tag="ew")
                sw = stat.tile([32, 1], F32, tag="sw")
                nc.scalar.activation(ew, Fm, AF.Exp, bias=mF, scale=-1.0,
                                     accum_out=sw)
                rw = stat.tile([32, 1], F32, tag="rw")
                nc.vector.reciprocal(rw, sw)
                wn = mp.tile([32, MH], F32, tag="wn")
                nc.vector.tensor_scalar_mul(wn, ew, rw)

                moa = mp.tile([32, D], F32, tag="moa")
                nc.vector.memset(moa, 0.0)
                for hh in range(MH):
                    nc.vector.scalar_tensor_tensor(
                        moa, in0=ret3[:, hh], scalar=wn[:, hh:hh + 1], in1=moa,
                        op0=ALU.mult, op1=ALU.add)

                # fe = mean(min_h F)
                feps = psO.tile([1, 1], F32, tag="po", name="feps")
                nc.tensor.matmul(feps, mF, ones32, start=True, stop=True)
                fesb = stat.tile([1, 1], F32, tag="fesb")
                nc.vector.tensor_copy(fesb, feps)
                nc.sync.dma_start(out=fe_ap, in_=fesb)

                mob = mp.tile([32, D], BF16, tag="cmb", name="mob")
                nc.vector.tensor_copy(mob, moa)
                moT = mp.tile([128, 8, 32], BF16, tag="moT")
                for dd in range(8):
                    transp(moT[:, dd], mob[:, ts(dd, 128)])
                wgo_o = wsmall.tile([128, 8, D], BF16, tag="wsmall", name="wgo_o")
                nc.sync.dma_start(out=wgo_o,
                                  in_=rearr_w("m_gw_ow")[:, 16:24, :])
                mo1 = mp.tile([32, D], F32, tag="ret3", name="mo1")
                for n in range(2):
                    ps = psA.tile([32, 512], F32, tag="ps", name=f"mo1ps{n}")
                    for dd in range(8):
                        nc.tensor.matmul(ps, moT[:, dd],
                                         wgo_o[:, dd, ts(n, 512)],
                                         start=(dd == 0), stop=(dd == 7))
                    bmo = mw.tile([32, 512], F32, tag="bb32", bufs=3, name=f"bmo{n}")
                    nc.sync.dma_start(out=bmo, in_=_bcast_row(
                        I["moutp_b"][:, ts(n, 512)], 32))
                    nc.vector.tensor_add(mo1[:, ts(n, 512)], ps, bmo)
                mo1b = mp.tile([32, D], BF16, tag="cmb", name="mo1b")
                nc.vector.tensor_copy(mo1b, mo1)
                moT2 = mp.tile([128, 8, 32], BF16, tag="moT", name="moT2")
                for dd in range(8):
                    transp(moT2[:, dd], mo1b[:, ts(dd, 128)])
                sg = mp.tile([32, D], F32, tag="sg")
                zga = mp.tile([32, D], F32, tag="moa", name="zga")
                wgo_a = wsmall.tile([128, 8, D], BF16, tag="wsmall", name="wgo_a")
                nc.sync.dma_start(out=wgo_a, in_=rearr_w("m_gw_ow")[:, 0:8, :])
                for n in range(2):
                    ps = psA.tile([32, 512], F32, tag="ps", name=f"sgpsA{n}")
                    for dd in range(8):
                        nc.tensor.matmul(ps, cmT[:, dd], wgo_a[:, dd, ts(n, 512)],
                                         start=(dd == 0), stop=(dd == 7))
                    bga = mw.tile([32, 512], F32, tag="bb32", bufs=3, name=f"bga{n}")
                    nc.sync.dma_start(out=bga, in_=_bcast_row(
                        I["mgate_b"][:, ts(n, 512)], 32))
                    nc.vector.tensor_add(zga[:, ts(n, 512)], ps, bga)
                wgo_b = wsmall.tile([128, 8, D], BF16, tag="wsmall", name="wgo_b")
                nc.sync.dma_start(out=wgo_b, in_=rearr_w("m_gw_ow")[:, 8:16, :])
                for n in range(2):
                    ps = psA.tile([32, 512], F32, tag="ps", name=f"sgpsB{n}")
                    for dd in range(8):
                        nc.tensor.matmul(ps, moT2[:, dd],
                                         wgo_b[:, dd, ts(n, 512)],
                                         start=(dd == 0), stop=(dd == 7))
                    nc.vector.tensor_add(ps, ps, zga[:, ts(n, 512)])
                    nc.scalar.activation(sg[:, ts(n, 512)], ps, AF.Sigmoid)
                dd_t = mw.tile([32, D], F32, tag="err", bufs=2, name="dd_t")
                nc.vector.tensor_sub(dd_t, mo1, cm32)
                tt_t = mw.tile([32, D], F32, tag="prec", bufs=2, name="tt_t")
                nc.vector.scalar_tensor_tensor(tt_t, in0=sg, scalar=0.3,
                                               in1=dd_t, op0=ALU.mult,
                                               op1=ALU.mult)
                mo_f = mp.tile([32, D], F32, tag="mo_f")
                nc.vector.tensor_add(mo_f, cm32, tt_t)
                nc.vector.tensor_sub(mo_f, mo_f, bline[:32])

                sel_sb = mp.tile([32, NT, 128], F32, tag="tgt", name="sel_sb")
                nc.sync.dma_start(out=sel_sb, in_=I["sel"].rearrange("t k m -> k t m"))
                for t in list(range(NT - 1, 0, -1)) + [0]:
                    for n in range(2):
                        ps = psA.tile([128, 512], F32, tag="ps",
                                      name=f"selps{t}{n}")
                        nc.tensor.matmul(ps, sel_sb[:, t], mo_f[:, ts(n, 512)],
                                         start=True, stop=True)
                        nc.vector.tensor_add(xbuf[:, t, ts(n, 512)], ps,
                                             xbuf[:, t, ts(n, 512)])

        # ---------------- schedule
        swa_block(0)
        halo_ag(0)
        swa_block(1)
        halo_ag(1)
        memory_stage()
        swa_block(2)
        halo_ag(2)
        swa_block(3)

        # ---------------- head
        with tc.tile_pool(name="headp", bufs=2) as hp:
            whd = wsmall.tile([128, 8, 256], BF16, tag="wsmall", name="whd")
            nc.sync.dma_start(out=whd, in_=rearr_w("head_w"))
            hbb = bbp.tile([128, 256], F32, tag="bbo", name="hbb")
            nc.sync.dma_start(out=hbb, in_=_bcast_row(I["head_b"], 128))
            for t in range(1, NT):
                rstd, nmr = ln_stats(xbuf[:, t])
                xh = work.tile([128, D], BF16, tag="xh")
                nc.scalar.activation(xh, xbuf[:, t], AF.Identity,
                                     bias=nmr, scale=rstd)
                xhTt = hp.tile([128, 8, 128], BF16, tag="xhTt")
                for dd in range(8):
                    transp(xhTt[:, dd], xh[:, ts(dd, 128)])
                ps = psA.tile([128, 256], F32, tag="ps")
                for dd in range(8):
                    nc.tensor.matmul(ps, xhTt[:, dd], whd[:, dd],
                                     start=(dd == 0), stop=(dd == 7))
                lg = work.tile([128, 256], F32, tag="lg")
                nc.vector.tensor_add(lg, ps, hbb)
                nc.sync.dma_start(out=logits_ap[ts(t - 1, 128), :], in_=lg)


# ---------------------------------------------------------------- host side
def _np32(x):
    return np.asarray(x, np.float32)


def _fold_ln(w, b_lin, g, b_ln):
    wg = _np32(w) * _np32(g)[:, None]
    bb = _np32(b_ln) @ _np32(w) + _np32(b_lin)
    return wg.astype(NPBF16), bb.astype(np.float32)


def _make_masks(is_even):
    x = np.arange(128)[:, None]
    y = np.arange(256)[None, :]
    band = (y >= x + 1) & (y <= x + 128)
    first = band & (y >= 128) if is_even else band
    m = np.zeros((2, 128, 256), np.float32)
    m[0][~first] = -1e30
    m[1][~band] = -1e30
    return m


def _prep_inputs(byte_ids, params):
    byte_ids = np.asarray(byte_ids)
    p = params
    emb = _np32(p["byte_emb"])
    ehi = emb.astype(NPBF16)
    elo = (emb - ehi.astype(np.float32)).astype(NPBF16)
    pos = _np32(p["pos_emb"])

    shared = {"emb_hi": ehi, "emb_lo": elo}
    blocks = list(p["lower"]) + list(p["upper"])
    for i, bp in enumerate(blocks):
        wq, bq = _fold_ln(bp["qkv_w"], bp["qkv_b"], bp["ln1_g"], bp["ln1_b"])
        wq = wq.astype(np.float32)
        wq[:, :D] *= 1.0 / 16.0          # fold 1/sqrt(DH) into q projection
        bq[:D] *= 1.0 / 16.0
        wq = wq.astype(NPBF16)
        w1, b1 = _fold_ln(bp["ff1_w"], bp["ff1_b"], bp["ln2_g"], bp["ln2_b"])
        b_v = bq[2 * D:]
        b_out = _np32(bp["out_b"]) + b_v @ _np32(bp["out_w"])
        shared[f"w_qkv{i}"] = wq
        shared[f"b_qkv{i}"] = bq[:2 * D].reshape(16, 128)
        shared[f"w_out{i}"] = _np32(bp["out_w"]).astype(NPBF16)
        shared[f"b_out{i}"] = b_out[None, :].astype(np.float32)
        shared[f"w_ff1{i}"] = w1
        shared[f"b_ff1{i}"] = b1.reshape(24, 128)
        shared[f"w_ff2{i}"] = _np32(bp["ff2_w"]).astype(NPBF16)
        shared[f"b_ff2{i}"] = _np32(bp["ff2_b"])[None, :].astype(np.float32)
    m = p["mem"]
    shared["mln_g"] = _np32(m["ln_g"])[None, :]
    shared["mln_b"] = _np32(m["ln_b"])[None, :]
    shared["mq_w"] = np.stack([_np32(h["q_w"]) for h in m["heads"]]).astype(NPBF16)
    shared["mq_b"] = np.concatenate(
        [_np32(h["q_b"]) for h in m["heads"]])[None, :].astype(np.float32)
    shared["mprec_w"] = np.concatenate(
        [_np32(h["prec_w"]) for h in m["heads"]], axis=1).astype(NPBF16)
    shared["mprec_b"] = np.concatenate(
        [_np32(h["prec_b"]) for h in m["heads"]])[None, :].astype(np.float32)
    shared["mS"] = np.concatenate(
        [_np32(h["S"]) for h in m["heads"]], axis=1).astype(NPBF16)
    shared["m_gw_ow"] = np.concatenate(
        [_np32(m["gate_w"]), _np32(m["outp_w"])], axis=0).astype(NPBF16)
    shared["mgate_b"] = _np32(m["gate_b"])[None, :]
    shared["moutp_b"] = _np32(m["outp_b"])[None, :]
    hw, hb = _fold_ln(p["head_w"], p["head_b"], p["lno_g"], p["lno_b"])
    shared["head_w"] = hw
    shared["head_b"] = hb[None, :]

    cones = np.zeros((8, 128, 4), np.float32)
    for t in range(8):
        cones[t, :, t // 2] = 1.0 / CS

    in_maps = []
    for c in range(NCORES):
        b, half = c // 2, c % 2
        s0 = half * TRES
        ids_pad = np.full(TALL, -1, np.int64)
        lo = max(0, s0 - 128)
        ids_pad[128 - (s0 - lo):] = byte_ids[b, lo:s0 + TRES]
        oh = np.zeros((256, TALL), np.float32)
        valid = ids_pad >= 0
        oh[ids_pad[valid], np.nonzero(valid)[0]] = 1.0
        posc = np.zeros((TALL, D), np.float32)
        posc[128 - (s0 - lo):] = pos[lo:s0 + TRES]
        sel = np.zeros((NT, 32, 128), np.float32)
        for t in range(NT):
            g = c * 4 + (t - 1) // 2
            if t == 0:
                g = c * 4 - 1 if half == 1 else 0
            sel[t, g, :] = 1.0
        m_ = {"oh": oh.astype(NPBF16), "pos": posc,
              "masks": _make_masks(half == 0), "sel": sel, "cones": cones}
        m_.update(shared)
        in_maps.append(m_)
    return in_maps


def kernel(byte_ids, params):
    if "nc" not in _CACHE:
        _CACHE["nc"] = _build()
    nc = _CACHE["nc"]
    in_maps = _prep_inputs(byte_ids, params)
    res = bass_utils.run_bass_kernel_spmd(nc, in_maps,
                                          core_ids=list(range(NCORES)))
    logits = np.zeros((B, S, 256), np.float32)
    for c in range(NCORES):
        b, half = c // 2, c % 2
        logits[b, half * TRES:(half + 1) * TRES] = res.results[c]["logits"]
    fe = np.float32(res.results[0]["fe"][0, 0])
    return logits, fe
